# revision 39
# baseline (speedup 1.0000x reference)
"""Trainium2 Bass kernel for nn_Graph_to_Featuremaps_savemem.

Reference computation:
    scores[b,p,n] = s_res[b,p] + s_hid[b,n];  attn = softmax_n(scores)
    out[b,c,p]    = relu(sum_n attn[b,p,n] * (x[b,n,:] @ W)[c])

Key simplification: softmax over n is shift-invariant, so the per-pixel
s_res[b,p] term (the only use of res_feature / node_fea_for_res) cancels:
    attn[b,p,n] = softmax_n(s_hid[b,n])   (independent of p)
    out[b,c,p]  = relu(sum_n a[b,n] * nv[b,n,c])  broadcast over all pixels.

So the kernel is a tiny softmax-weighted matmul (per-batch (7,256)x(256,256))
followed by an 18.9 MB-per-core broadcast-write of the (BL,C) result over
H*W pixels.  Sharding: data-parallel over batch, 2 batches per core across
8 cores; the small params (node_fea_for_hidden, weight) are replicated.

The store is the roofline: ~44.5 us of DMA at the ~424 GB/s per-core cap
(16 engines x 26.5 GB/s).  Everything else is head/tail latency, minimized
as follows:
- x is packed host-side already transposed (h-major), so no PE transposes
  or PSUM round-trips before the first matmul; all matmul operands come
  straight out of the single packed input tile.
- Matmul operands are bitcast to float32r, halving the PE pass count
  (fp32 matmuls otherwise run LOW+HIGH double passes).
- Softmax is computed in column layout ((b,n) on partitions): exp on ACT,
  per-batch denominators via a block-diagonal matmul, reciprocal+scale on
  DVE.  The attn column never needs transposing.
- The broadcast fill (DVE, ~950 GB/s) and the store DMA (~424 GB/s) are
  pipelined in chunks along the pixel axis, with small first chunks so the
  first store triggers right after a ~0.6 MB fill instead of after a full
  9.4 MB batch fill.  Chunks grow geometrically so the store queue never
  idles once started.

Hardware constraints shaping the structure:
- Walrus accepts a single sync wait per instruction, so every cross-engine
  semaphore is acquired once by a copy with a free wait slot and later
  instructions are covered transitively; each PE-matmul / tensor-scalar /
  DMA-trigger operand pair shares one producer semaphore (cin+cin from the
  input queue, e_col+blk2 both ACT, sb_nv+ablk both DVE).  For
  TensorScalarPtr both in0 and the ptr operand must be DVE-produced.
- Tile allocates only 8 DMAHW semaphores and reusing one puts a second
  sync wait on the trigger: 1 input DMA + 7 stores = 8 total.
- The input DMA triggers from SP (earliest preamble exit); the stores all
  share the Scalar engine's hardware queue so each DMA engine drains them
  in FIFO order.
- The kernel-tail drain also has limited wait slots; _fix_tail_drain
  strips it to the one semaphore whose completion implies all others
  (the last store's queue sem).
"""

import numpy as np

import concourse.bass as bass
import concourse.mybir as mybir
import concourse.tile as tile
from concourse.alu_op_type import AluOpType
from concourse.bass_utils import run_bass_kernel_spmd

B, NODES, HID, C, H, W = 16, 7, 256, 256, 96, 96
P = H * W                # 9216 pixels
NCORES = 8
BL = B // NCORES         # 2 local batches per core
BN = BL * NODES          # 14 (b,n) rows

# Pixel-axis chunking of the fill->store pipeline (7 stores total).  The
# first C0 pixels of batch 0 ship as TWO single-channel stores so the very
# first trigger waits on one broadcast fill instead of two.
C0 = 576
CHUNKS = {
    0: [1152, 3456, 4032],
    1: [4608, 4608],
}
assert C0 + sum(CHUNKS[0]) == P and sum(CHUNKS[1]) == P

# Packed input layout: (128, CIN_COLS) float32
COL_W = 0         # cols 0:512, all rows: w[kh*128+k, c] at [k, kh*256+c]
COL_XT = 512      # cols 512:540: xT[k, kh*BN + (b n)] = x[(b n), kh*128+k]
COL_NFH = 540     # cols 540:542: nfh[kh*128+k] at [k, kh]
COL_BLK2 = 542    # cols 542:556, rows 0:14: block-diag ones(7,7) x2
COL_BM = 556      # cols 556:558, rows 0:14: block mask [(b n), b]
CIN_COLS = 558

_cache: dict = {}


def _build_nc():
    nc = bass.Bass()
    dt = mybir.dt.float32
    dtr = mybir.dt.float32r
    dtb = mybir.dt.bfloat16
    # cin is bf16 end-to-end: halves the input DMA wire time and lets the
    # s/nv matmuls run single-pass natively.  The mask sections are 0/1
    # (exact in bf16); fp32 consumers use converting copies.
    cin_d = nc.declare_dram_parameter("cin", [128, CIN_COLS], dtb, isOutput=False)
    out_d = nc.declare_dram_parameter("out", [BL, C, P], dt, isOutput=True)

    with tile.TileContext(nc) as tc:
        with (
            tc.tile_pool(name="sb", bufs=1) as sb,
            tc.tile_pool(name="ps", bufs=1, space=bass.MemorySpace.PSUM) as ps,
        ):
            cin = sb.tile([128, CIN_COLS], dtb)
            # SP's preamble finishes ~0.45us before Scalar's, so it triggers
            # the input DMA earlier (the E79 straggle once attributed to a
            # second queue turned out to be bursty neighbor noise).
            nc.sync.dma_start(out=cin[:], in_=cin_d[:])

            xt = [cin[:, COL_XT + kh * BN : COL_XT + (kh + 1) * BN] for kh in (0, 1)]
            nfh = [cin[:, COL_NFH + kh : COL_NFH + kh + 1] for kh in (0, 1)]
            wh = [cin[:, kh * C : (kh + 1) * C] for kh in (0, 1)]
            blkmask = cin[0:BN, COL_BM : COL_BM + BL]

            # ACT copies blk2 so the denominator matmul's operands (e_col,
            # blk2) share the ACT semaphore.
            sc_blk2 = sb.tile([BN, BN], dt)
            nc.scalar.copy(
                out=sc_blk2[:],
                in_=cin[0:BN, COL_BLK2 : COL_BLK2 + BN],
            )

            # s_col[(b n), 0] = sum_h x[(b n), h] * nfh[h]; nv = x @ W.
            # PE order matters (in-order engine): nv is the long pole and its
            # operands are ready first; den waits on exp which is ready long
            # before nv finishes.
            ps_s = ps.tile([BN, 1], dt, tag="s")
            ps_nv = ps.tile([BN, C], dt, tag="nv")
            for kh in range(2):
                nc.tensor.matmul(
                    ps_s[:], xt[kh], nfh[kh], start=(kh == 0), stop=(kh == 1)
                )
            for kh in range(2):
                nc.tensor.matmul(
                    ps_nv[:], xt[kh], wh[kh], start=(kh == 0), stop=(kh == 1)
                )

            # Softmax over the 7 nodes of each local batch, in column layout.
            e_col = sb.tile([BN, 1], dt)
            nc.scalar.activation(e_col[:], ps_s[:], mybir.ActivationFunctionType.Exp)
            ps_den = ps.tile([BN, 1], dt, tag="den")
            nc.tensor.matmul(ps_den[:], sc_blk2[:], e_col[:], start=True, stop=True)

            # DVE stream.  sb_nv first (its producer nv finishes before den),
            # then the softmax-scale chain.
            sb_bm = sb.tile([BN, BL], dt)
            nc.vector.tensor_copy(out=sb_bm[:], in_=blkmask)       # DMAHW wait
            sb_e = sb.tile([BN, 1], dt)
            nc.vector.tensor_copy(out=sb_e[:], in_=e_col[:])       # ACT wait
            # sb_nv / ablk are written as float32r (rounding copies) so the
            # v matmul can also run single-pass.
            sb_nv = sb.tile([BN, C], dtr)
            nc.vector.tensor_copy(out=sb_nv[:], in_=ps_nv[:])      # PE wait
            recip = sb.tile([BN, 1], dt)
            nc.vector.reciprocal(recip[:], ps_den[:])              # PE wait
            # ablk[(b n), b'] = attn[(b n)] if b == b' else 0, in one
            # scalar_tensor_tensor: (blkmask * recip) * e_broadcast.
            ablk = sb.tile([BN, BL], dtr)
            nc.vector.scalar_tensor_tensor(
                ablk[:],
                sb_bm[:],
                recip[:],
                sb_e[:].to_broadcast([BN, BL]),
                AluOpType.mult,
                AluOpType.mult,
            )

            # v[c, ch*BL + b] = sum_n attn[b, n] * nv[(b n), c], c = ch*128+p.
            ps_v = ps.tile([128, 2 * BL], dt, tag="v")
            for ch in range(2):
                nc.tensor.matmul(
                    ps_v[:, ch * BL : (ch + 1) * BL],
                    sb_nv[:, ch * 128 : (ch + 1) * 128],
                    ablk[:],
                    start=True,
                    stop=True,
                )
            sb_v = sb.tile([128, 2 * BL], dt)
            nc.scalar.activation(sb_v[:], ps_v[:], mybir.ActivationFunctionType.Relu)

            # Pipelined broadcast fill (DVE) -> store (DMA) over pixel chunks.
            # Fills read the relu'd sb_v from SBUF: DVE's PSUM-read path is
            # ~2x slower and cannot keep the store queue fed.
            # First two stores: single-channel slices of batch 0, pixel 0:C0.
            # c = ch*128+p, so a single-channel store is a plain 2D AP.
            for ch in range(2):
                bc0 = sb.tile([128, C0], dt, tag=f"bc0s_{ch}")
                nc.vector.tensor_copy(
                    out=bc0[:],
                    in_=sb_v[:, ch * BL : ch * BL + 1].to_broadcast([128, C0]),
                )
                nc.scalar.dma_start(
                    out=out_d[0, ch * 128 : (ch + 1) * 128, 0:C0], in_=bc0[:]
                )
            # out[b] is (256, P) in DRAM, viewed as [p, ch, pix], c = ch*128+p.
            for b in range(BL):
                o = C0 if b == 0 else 0
                for k, chw in enumerate(CHUNKS[b]):
                    bc = sb.tile([128, 2, chw], dt, tag=f"bc{b}_{k}")
                    for ch in range(2):
                        j = ch * BL + b
                        nc.vector.tensor_copy(
                            out=bc[:, ch, :],
                            in_=sb_v[:, j : j + 1].to_broadcast([128, chw]),
                        )
                    nc.scalar.dma_start(
                        out=out_d[b, :, o : o + chw].rearrange(
                            "(ch p) pix -> p ch pix", p=128
                        ),
                        in_=bc[:],
                    )
                    o += chw
    _fix_tail_drain(nc)
    return nc


def _fix_tail_drain(nc):
    """Walrus in this toolchain accepts very few sync waits per instruction,
    and Tile's kernel-tail drain waits on every semaphore.  The dataflow is
    one chain ending in the store DMAs, which all share one hardware queue:
    each of the queue's engines processes its descriptors in FIFO order, so
    the LAST store's completion semaphore reaching its target implies every
    earlier packet (and everything upstream of the triggers) is done.  Strip
    the drain down to that one wait."""
    import bass_rust

    out_sem = None
    for ins in nc.inst_map.values():
        if type(ins).__name__ == "InstDMACopy" and "out_set" in str(ins):
            si = ins.sync_info
            if si is not None and len(si.on_update) > 0:
                out_sem = si.on_update[0].ant_name
    assert out_sem is not None, "output DMA completion sem not found"
    for ins in nc.inst_map.values():
        si = ins.sync_info
        if type(ins).__name__ == "InstDrain" and si is not None and len(si.on_wait) > 1:
            keep = [w for w in si.on_wait if w.ant_name == out_sem]
            assert len(keep) == 1, (out_sem, [w.ant_name for w in si.on_wait])
            ins.sync_info = bass_rust.SyncInfo(
                on_wait=keep, on_update=list(si.on_update)
            )


def _get_nc():
    if "nc" not in _cache:
        _cache["nc"] = _build_nc()
    return _cache["nc"]


def _pack_cin(x_shard, nfh, w):
    """Pack one core's inputs into the (128, CIN_COLS) bf16 tensor."""
    import ml_dtypes

    cin = np.zeros((128, CIN_COLS), dtype=np.float32)
    x2 = x_shard.reshape(BN, HID)
    for kh in range(2):
        cin[:, kh * C : (kh + 1) * C] = w[kh * 128 : (kh + 1) * 128, :]
        cin[:, COL_XT + kh * BN : COL_XT + (kh + 1) * BN] = x2[
            :, kh * 128 : (kh + 1) * 128
        ].T
        cin[:, COL_NFH + kh] = nfh[kh * 128 : (kh + 1) * 128, 0]
    for b in range(BL):
        rr = slice(b * NODES, (b + 1) * NODES)
        cin[rr, COL_BLK2 + b * NODES : COL_BLK2 + (b + 1) * NODES] = 1.0
        cin[rr, COL_BM + b] = 1.0
    return cin.astype(ml_dtypes.bfloat16)


def _make_in_maps(input, node_fea_for_hidden, weight):
    x_full = np.asarray(input, dtype=np.float32)[0]  # (B, N, HID)
    nfh = np.asarray(node_fea_for_hidden, dtype=np.float32)
    w = np.asarray(weight, dtype=np.float32)
    return [
        {"cin": _pack_cin(x_full[i * BL : (i + 1) * BL], nfh, w)}
        for i in range(NCORES)
    ]


def _run(in_maps, trace=False, **kwargs):
    nc = _get_nc()
    return run_bass_kernel_spmd(nc, in_maps, list(range(NCORES)), trace=trace, **kwargs)


def kernel(input, res_feature, node_fea_for_res, node_fea_for_hidden, weight):
    in_maps = _make_in_maps(input, node_fea_for_hidden, weight)
    res = _run(in_maps)
    shards = [res.results[i]["out"] for i in range(NCORES)]  # each (BL, C, P)
    full = np.concatenate(shards, axis=0)  # (B, C, P)
    return full.reshape(B, C, H, W).astype(np.float32, copy=False)


# revision 45
# speedup vs baseline: 1.0117x; 1.0117x over previous
"""Trainium2 Bass kernel for nn_Graph_to_Featuremaps_savemem.

Reference computation:
    scores[b,p,n] = s_res[b,p] + s_hid[b,n];  attn = softmax_n(scores)
    out[b,c,p]    = relu(sum_n attn[b,p,n] * (x[b,n,:] @ W)[c])

Key simplification: softmax over n is shift-invariant, so the per-pixel
s_res[b,p] term (the only use of res_feature / node_fea_for_res) cancels:
    attn[b,p,n] = softmax_n(s_hid[b,n])   (independent of p)
    out[b,c,p]  = relu(sum_n a[b,n] * nv[b,n,c])  broadcast over all pixels.

So the kernel is a tiny softmax-weighted matmul (per-batch (7,256)x(256,256))
followed by an 18.9 MB-per-core broadcast-write of the (BL,C) result over
H*W pixels.  Sharding: data-parallel over batch, 2 batches per core across
8 cores; the small params (node_fea_for_hidden, weight) are replicated.

The store is the roofline: ~44.5 us of DMA at the ~424 GB/s per-core cap
(16 engines x 26.5 GB/s).  Everything else is head/tail latency, minimized
as follows:
- x is packed host-side already transposed (h-major), so no PE transposes
  or PSUM round-trips before the first matmul; all matmul operands come
  straight out of the single packed input tile.
- Matmul operands are bitcast to float32r, halving the PE pass count
  (fp32 matmuls otherwise run LOW+HIGH double passes).
- Softmax is computed in column layout ((b,n) on partitions): exp on ACT,
  per-batch denominators via a block-diagonal matmul, reciprocal+scale on
  DVE.  The attn column never needs transposing.
- The broadcast fill (DVE, ~950 GB/s) and the store DMA (~424 GB/s) are
  pipelined in chunks along the pixel axis, with small first chunks so the
  first store triggers right after a ~0.6 MB fill instead of after a full
  9.4 MB batch fill.  Chunks grow geometrically so the store queue never
  idles once started.

Hardware constraints shaping the structure:
- Walrus accepts a single sync wait per instruction, so every cross-engine
  semaphore is acquired once by a copy with a free wait slot and later
  instructions are covered transitively; each PE-matmul / tensor-scalar /
  DMA-trigger operand pair shares one producer semaphore (cin+cin from the
  input queue, e_col+blk2 both ACT, sb_nv+ablk both DVE).  For
  TensorScalarPtr both in0 and the ptr operand must be DVE-produced.
- Tile allocates only 8 DMAHW semaphores and reusing one puts a second
  sync wait on the trigger: 1 input DMA + 7 stores = 8 total.
- The input DMA triggers from SP (earliest preamble exit); the stores all
  share the Scalar engine's hardware queue so each DMA engine drains them
  in FIFO order.
- The kernel-tail drain also has limited wait slots; _fix_tail_drain
  strips it to the one semaphore whose completion implies all others
  (the last store's queue sem).
"""

import numpy as np

import concourse.bass as bass
import concourse.mybir as mybir
import concourse.tile as tile
from concourse.alu_op_type import AluOpType
from concourse.bass_utils import run_bass_kernel_spmd

B, NODES, HID, C, H, W = 16, 7, 256, 256, 96, 96
P = H * W                # 9216 pixels
NCORES = 8
BL = B // NCORES         # 2 local batches per core
BN = BL * NODES          # 14 (b,n) rows

# Pixel-axis chunking of the fill->store pipeline (7 stores total).  The
# first C0 pixels of batch 0 ship as TWO single-channel stores so the very
# first trigger waits on one broadcast fill instead of two.
C0 = 576
CHUNKS = {
    0: [1152, 3456, 4032],
    1: [4608, 4608],
}
assert C0 + sum(CHUNKS[0]) == P and sum(CHUNKS[1]) == P

# Packed input layout: (128, CIN_COLS) float32
COL_W = 0         # cols 0:512, all rows: w[kh*128+k, c] at [k, kh*256+c]
COL_XT = 512      # cols 512:540: xT[k, kh*BN + (b n)] = x[(b n), kh*128+k]
COL_NFH = 540     # cols 540:542: nfh[kh*128+k] at [k, kh]
COL_BLK2 = 542    # cols 542:556, rows 0:14: block-diag ones(7,7) x2
COL_BM = 556      # cols 556:558, rows 0:14: block mask [(b n), b]
CIN_COLS = 558

_cache: dict = {}


def _build_nc():
    nc = bass.Bass()
    dt = mybir.dt.float32
    dtr = mybir.dt.float32r
    dtb = mybir.dt.bfloat16
    # cin is bf16 end-to-end: halves the input DMA wire time and lets the
    # s/nv matmuls run single-pass natively.  The mask sections are 0/1
    # (exact in bf16); fp32 consumers use converting copies.
    cin_d = nc.declare_dram_parameter("cin", [128, CIN_COLS], dtb, isOutput=False)
    out_d = nc.declare_dram_parameter("out", [BL, C, P], dt, isOutput=True)

    with tile.TileContext(nc) as tc:
        with (
            tc.tile_pool(name="sb", bufs=1) as sb,
            tc.tile_pool(name="ps", bufs=1, space=bass.MemorySpace.PSUM) as ps,
        ):
            cin = sb.tile([128, CIN_COLS], dtb)
            # SP's preamble finishes ~0.45us before Scalar's, so it triggers
            # the input DMA earlier (the E79 straggle once attributed to a
            # second queue turned out to be bursty neighbor noise).
            nc.sync.dma_start(out=cin[:], in_=cin_d[:])

            xt = [cin[:, COL_XT + kh * BN : COL_XT + (kh + 1) * BN] for kh in (0, 1)]
            nfh = [cin[:, COL_NFH + kh : COL_NFH + kh + 1] for kh in (0, 1)]
            wh = [cin[:, kh * C : (kh + 1) * C] for kh in (0, 1)]
            blkmask = cin[0:BN, COL_BM : COL_BM + BL]

            # ACT copies blk2 so the denominator matmul's operands (e_col,
            # blk2) share the ACT semaphore.
            sc_blk2 = sb.tile([BN, BN], dt)
            nc.scalar.copy(
                out=sc_blk2[:],
                in_=cin[0:BN, COL_BLK2 : COL_BLK2 + BN],
            )

            # s_col[(b n), 0] = sum_h x[(b n), h] * nfh[h]; nv = x @ W.
            # PE order matters (in-order engine): nv is the long pole and its
            # operands are ready first; den waits on exp which is ready long
            # before nv finishes.
            ps_s = ps.tile([BN, 1], dt, tag="s")
            ps_nv = ps.tile([BN, C], dt, tag="nv")
            for kh in range(2):
                nc.tensor.matmul(
                    ps_s[:], xt[kh], nfh[kh], start=(kh == 0), stop=(kh == 1)
                )
            for kh in range(2):
                nc.tensor.matmul(
                    ps_nv[:], xt[kh], wh[kh], start=(kh == 0), stop=(kh == 1)
                )

            # Softmax over the 7 nodes of each local batch, in column layout.
            e_col = sb.tile([BN, 1], dt)
            nc.scalar.activation(e_col[:], ps_s[:], mybir.ActivationFunctionType.Exp)
            ps_den = ps.tile([BN, 1], dt, tag="den")
            nc.tensor.matmul(ps_den[:], sc_blk2[:], e_col[:], start=True, stop=True)

            # DVE stream, interleaved so ablk and the ch0 nv cast finish
            # together: the ch0 v matmul (and so the first store) waits on
            # max(cast0, ablk).  sb_nv / ablk are written as float32r
            # (rounding copies) so the v matmul also runs single-pass.
            sb_bm = sb.tile([BN, BL], dt)
            nc.vector.tensor_copy(out=sb_bm[:], in_=blkmask)       # DMAHW wait
            sb_e = sb.tile([BN, 1], dt)
            nc.vector.tensor_copy(out=sb_e[:], in_=e_col[:])       # ACT wait
            sb_nv0 = sb.tile([BN, 128], dtr, tag="nv0")
            sb_nv1 = sb.tile([BN, 128], dtr, tag="nv1")
            sb_nv = [sb_nv0, sb_nv1]
            nc.vector.tensor_copy(out=sb_nv[0][:], in_=ps_nv[:, 0:128])  # PE wait
            recip = sb.tile([BN, 1], dt)
            nc.vector.reciprocal(recip[:], ps_den[:])              # PE wait
            # ablk[(b n), b'] = attn[(b n)] if b == b' else 0, in one
            # scalar_tensor_tensor: (blkmask * recip) * e_broadcast.
            ablk = sb.tile([BN, BL], dtr)
            nc.vector.scalar_tensor_tensor(
                ablk[:],
                sb_bm[:],
                recip[:],
                sb_e[:].to_broadcast([BN, BL]),
                AluOpType.mult,
                AluOpType.mult,
            )
            nc.vector.tensor_copy(out=sb_nv[1][:], in_=ps_nv[:, 128:256])

            # v[c, ch*BL + b] = sum_n attn[b, n] * nv[(b n), c], c = ch*128+p.
            # Per-channel v matmul and relu, so the ch0 store path never
            # waits on ch1 work.
            ps_v0 = ps.tile([128, BL], dt, tag="pv0")
            ps_v1 = ps.tile([128, BL], dt, tag="pv1")
            ps_v = [ps_v0, ps_v1]
            sb_v0 = sb.tile([128, BL], dt, tag="v0")
            sb_v1 = sb.tile([128, BL], dt, tag="v1")
            sb_v = [sb_v0, sb_v1]
            for ch in range(2):
                nc.tensor.matmul(
                    ps_v[ch][:],
                    sb_nv[ch][:],
                    ablk[:],
                    start=True,
                    stop=True,
                )
                nc.scalar.activation(
                    sb_v[ch][:],
                    ps_v[ch][:],
                    mybir.ActivationFunctionType.Relu,
                )

            # Pipelined broadcast fill (DVE) -> store (DMA) over pixel chunks.
            # Fills read the relu'd sb_v from SBUF: DVE's PSUM-read path is
            # ~2x slower and cannot keep the store queue fed.
            # First two stores: single-channel slices of batch 0, pixel 0:C0.
            # c = ch*128+p, so a single-channel store is a plain 2D AP.
            for ch in range(2):
                bc0 = sb.tile([128, C0], dt, tag=f"bc0s_{ch}")
                nc.vector.tensor_copy(
                    out=bc0[:],
                    in_=sb_v[ch][:, 0:1].to_broadcast([128, C0]),
                )
                nc.scalar.dma_start(
                    out=out_d[0, ch * 128 : (ch + 1) * 128, 0:C0], in_=bc0[:]
                )
            # out[b] is (256, P) in DRAM, viewed as [p, ch, pix], c = ch*128+p.
            for b in range(BL):
                o = C0 if b == 0 else 0
                for k, chw in enumerate(CHUNKS[b]):
                    bc = sb.tile([128, 2, chw], dt, tag=f"bc{b}_{k}")
                    for ch in range(2):
                        nc.vector.tensor_copy(
                            out=bc[:, ch, :],
                            in_=sb_v[ch][:, b : b + 1].to_broadcast([128, chw]),
                        )
                    nc.scalar.dma_start(
                        out=out_d[b, :, o : o + chw].rearrange(
                            "(ch p) pix -> p ch pix", p=128
                        ),
                        in_=bc[:],
                    )
                    o += chw
    _fix_tail_drain(nc)
    return nc


def _fix_tail_drain(nc):
    """Walrus in this toolchain accepts very few sync waits per instruction,
    and Tile's kernel-tail drain waits on every semaphore.  The dataflow is
    one chain ending in the store DMAs, which all share one hardware queue:
    each of the queue's engines processes its descriptors in FIFO order, so
    the LAST store's completion semaphore reaching its target implies every
    earlier packet (and everything upstream of the triggers) is done.  Strip
    the drain down to that one wait."""
    import bass_rust

    out_sem = None
    for ins in nc.inst_map.values():
        if type(ins).__name__ == "InstDMACopy" and "out_set" in str(ins):
            si = ins.sync_info
            if si is not None and len(si.on_update) > 0:
                out_sem = si.on_update[0].ant_name
    assert out_sem is not None, "output DMA completion sem not found"
    for ins in nc.inst_map.values():
        si = ins.sync_info
        if type(ins).__name__ == "InstDrain" and si is not None and len(si.on_wait) > 1:
            keep = [w for w in si.on_wait if w.ant_name == out_sem]
            assert len(keep) == 1, (out_sem, [w.ant_name for w in si.on_wait])
            ins.sync_info = bass_rust.SyncInfo(
                on_wait=keep, on_update=list(si.on_update)
            )


def _get_nc():
    if "nc" not in _cache:
        _cache["nc"] = _build_nc()
    return _cache["nc"]


def _pack_cin(x_shard, nfh, w):
    """Pack one core's inputs into the (128, CIN_COLS) bf16 tensor."""
    import ml_dtypes

    cin = np.zeros((128, CIN_COLS), dtype=np.float32)
    x2 = x_shard.reshape(BN, HID)
    for kh in range(2):
        cin[:, kh * C : (kh + 1) * C] = w[kh * 128 : (kh + 1) * 128, :]
        cin[:, COL_XT + kh * BN : COL_XT + (kh + 1) * BN] = x2[
            :, kh * 128 : (kh + 1) * 128
        ].T
        cin[:, COL_NFH + kh] = nfh[kh * 128 : (kh + 1) * 128, 0]
    for b in range(BL):
        rr = slice(b * NODES, (b + 1) * NODES)
        cin[rr, COL_BLK2 + b * NODES : COL_BLK2 + (b + 1) * NODES] = 1.0
        cin[rr, COL_BM + b] = 1.0
    return cin.astype(ml_dtypes.bfloat16)


def _make_in_maps(input, node_fea_for_hidden, weight):
    x_full = np.asarray(input, dtype=np.float32)[0]  # (B, N, HID)
    nfh = np.asarray(node_fea_for_hidden, dtype=np.float32)
    w = np.asarray(weight, dtype=np.float32)
    return [
        {"cin": _pack_cin(x_full[i * BL : (i + 1) * BL], nfh, w)}
        for i in range(NCORES)
    ]


def _run(in_maps, trace=False, **kwargs):
    nc = _get_nc()
    return run_bass_kernel_spmd(nc, in_maps, list(range(NCORES)), trace=trace, **kwargs)


def kernel(input, res_feature, node_fea_for_res, node_fea_for_hidden, weight):
    in_maps = _make_in_maps(input, node_fea_for_hidden, weight)
    res = _run(in_maps)
    shards = [res.results[i]["out"] for i in range(NCORES)]  # each (BL, C, P)
    full = np.concatenate(shards, axis=0)  # (B, C, P)
    return full.reshape(B, C, H, W).astype(np.float32, copy=False)
